# revision 1
# baseline (speedup 1.0000x reference)
"""BartCustomAttention Trainium2 kernel.

Sharding: 8 cores = batch(2) x t-block(4, 256 rows each). Each core computes
all 16 heads for its 256 query rows; k/v projections for its batch element are
computed redundantly on the 4 cores sharing it (cheaper than cross-core
exchange at this size).

Relation-value term: out2[h,t,:] = sum_s attn[h,t,s] * E[r[t,s],:]
  = W[h,t,:41] @ E, with W the attn-weighted histogram of relation codes.
W is computed on the tensor engine: per (t, s-chunk) a [128s,16h]x[128s,42]
matmul against a one-hot(+ones) matrix built on the vector engine; 4 t's are
packed per PSUM tile via column tiling. The W @ (E @ Wo_h.T) product is folded
into the output projection via a host-precomputed packed weight.

Softmax skips the max-subtraction (scores are O(5) for this distribution;
mathematically identical). Normalization by Z = sum(exp) happens at the end:
Z rows come from a ones-vector matmul over the transposed probabilities, and
the per-(h,t) 1/Z broadcast tile is built with a K=1 outer-product matmul.
"""

import sys

if "/opt/trn_rl_repo" not in sys.path:
    sys.path.insert(0, "/opt/trn_rl_repo")

import numpy as np
import ml_dtypes

import concourse.bass as bass
from concourse import bacc
import concourse.mybir as mybir
import concourse.tile as tile
from concourse import bass_utils

B, T, D, H, DH = 2, 1024, 1024, 16, 64
NJ = 42  # 41 relation bins + ones column (gives Z for free, unused now)
TB = T // 4  # 256 query rows per core
P = 128
N_CORES = 8
TBATCH = 16  # t's per one-hot build batch

F32 = mybir.dt.float32
BF16 = mybir.dt.bfloat16
I32 = mybir.dt.int32


def build_bass():
    nc = bacc.Bacc(None, target_bir_lowering=False)

    hsT = nc.dram_tensor("hsT", [D, T], BF16, kind="ExternalInput")
    hsTq = nc.dram_tensor("hsTq", [D, TB], BF16, kind="ExternalInput")
    rT = nc.dram_tensor("rT", [T, TB], BF16, kind="ExternalInput")
    WqT = nc.dram_tensor("WqT", [D, D], BF16, kind="ExternalInput")
    WkT = nc.dram_tensor("WkT", [D, D], BF16, kind="ExternalInput")
    WvT = nc.dram_tensor("WvT", [D, D], BF16, kind="ExternalInput")
    WoP = nc.dram_tensor("WoP", [H, P, D], BF16, kind="ExternalInput")
    bqk = nc.dram_tensor("bqk", [T, H], F32, kind="ExternalInput")
    bvT = nc.dram_tensor("bvT", [DH, H], BF16, kind="ExternalInput")
    out = nc.dram_tensor("out", [TB, D], F32, kind="ExternalOutput")

    with tile.TileContext(nc) as tc:
        with (
            tc.tile_pool(name="persist", bufs=1) as persist,
            tc.tile_pool(name="psProj", bufs=2, space="PSUM") as psProj,
            tc.tile_pool(name="psSc", bufs=2, space="PSUM") as psSc,
            tc.tile_pool(name="psSmall", bufs=1, space="PSUM") as psSmall,
            tc.tile_pool(name="psW", bufs=1, space="PSUM") as psW,
        ):
            # ---- persistent small inputs + big activations ----
            rTs = persist.tile([P, 8, TB], BF16)
            nc.sync.dma_start(rTs[:], rT.rearrange("(sc p) t -> p sc t", p=P))
            bqks = persist.tile([P, 8, H], F32)
            nc.sync.dma_start(bqks[:], bqk.rearrange("(sc p) h -> p sc h", p=P))
            bvs = persist.tile([DH, H], BF16)
            nc.sync.dma_start(bvs[:], bvT[:, :])

            iotaI = persist.tile([P, 41], I32)
            nc.gpsimd.iota(iotaI[:], pattern=[[1, 41]], base=0, channel_multiplier=0)
            iotaF = persist.tile([P, TBATCH, 41], BF16)
            nc.vector.tensor_copy(
                out=iotaF[:], in_=iotaI[:, None, :].to_broadcast([P, TBATCH, 41])
            )
            onescol = persist.tile([P, 1], BF16)
            nc.vector.memset(onescol[:], 1.0)
            onesrow = persist.tile([1, P], F32)
            nc.vector.memset(onesrow[:], 1.0)

            AT = persist.tile([P, 8, TB, H], BF16)
            oT = persist.tile([P, TB, H], BF16)
            recipZ = persist.tile([1, H, TB], F32)
            # rows 96-104 are re-written by the W assembly DMA later; row 105 is
            # the constant-1 row that carries bo through the fused projection.
            nc.vector.memset(oT[96:128, :, :], 0.0)
            onesbig = persist.tile([1, TB * H], BF16)
            nc.vector.memset(onesbig[:], 1.0)
            nc.sync.dma_start(
                out=oT[105:106, :, :].rearrange("p t h -> p (t h)"),
                in_=onesbig[:],
            )

            with tc.tile_pool(name="acts", bufs=1) as acts:
                kT = acts.tile([P, 8, T], BF16)  # [d_model rows, ., s]
                vS = acts.tile([P, 8, D], BF16)  # [s rows, ., d_model]
                qT = acts.tile([P, 8, TB], BF16)

                with tc.tile_pool(name="win", bufs=1) as win:
                    hsTs = win.tile([P, 8, T], BF16)
                    nc.sync.dma_start(
                        hsTs[:], hsT.rearrange("(ic p) s -> p ic s", p=P)
                    )
                    hsTqs = win.tile([P, 8, TB], BF16)
                    nc.sync.dma_start(
                        hsTqs[:], hsTq.rearrange("(ic p) t -> p ic t", p=P)
                    )
                    Wq_s = win.tile([P, 8, D], BF16)
                    nc.sync.dma_start(Wq_s[:], WqT.rearrange("(ic p) o -> p ic o", p=P))
                    Wk_s = win.tile([P, 8, D], BF16)
                    nc.sync.dma_start(Wk_s[:], WkT.rearrange("(ic p) o -> p ic o", p=P))
                    Wv_s = win.tile([P, 8, D], BF16)
                    nc.sync.dma_start(Wv_s[:], WvT.rearrange("(ic p) o -> p ic o", p=P))

                    # ---- phase 1: projections ----
                    for oc in range(8):
                        for n in range(2):
                            ps = psProj.tile([P, 512], F32, tag="proj")
                            for ic in range(8):
                                nc.tensor.matmul(
                                    ps[:],
                                    lhsT=Wk_s[:, ic, oc * P : (oc + 1) * P],
                                    rhs=hsTs[:, ic, n * 512 : (n + 1) * 512],
                                    start=(ic == 0),
                                    stop=(ic == 7),
                                )
                            nc.vector.tensor_copy(
                                out=kT[:, oc, n * 512 : (n + 1) * 512], in_=ps[:]
                            )
                    for sc in range(8):
                        for n in range(2):
                            ps = psProj.tile([P, 512], F32, tag="proj")
                            for ic in range(8):
                                nc.tensor.matmul(
                                    ps[:],
                                    lhsT=hsTs[:, ic, sc * P : (sc + 1) * P],
                                    rhs=Wv_s[:, ic, n * 512 : (n + 1) * 512],
                                    start=(ic == 0),
                                    stop=(ic == 7),
                                )
                            nc.vector.tensor_copy(
                                out=vS[:, sc, n * 512 : (n + 1) * 512], in_=ps[:]
                            )
                    for oc in range(8):
                        ps = psProj.tile([P, 512], F32, tag="proj")
                        for ic in range(8):
                            nc.tensor.matmul(
                                ps[:, :TB],
                                lhsT=Wq_s[:, ic, oc * P : (oc + 1) * P],
                                rhs=hsTqs[:, ic, :],
                                start=(ic == 0),
                                stop=(ic == 7),
                            )
                        nc.vector.tensor_copy(out=qT[:, oc, :], in_=ps[:, :TB])

                # ---- phase 2a: scoresT -> exp -> AT ----
                for h in range(H):
                    base = (h % 2) * 64
                    oc = h // 2
                    for sc in range(8):
                        ps = psSc.tile([P, TB], F32, tag="scoresT")
                        nc.tensor.matmul(
                            ps[:],
                            lhsT=kT[base : base + 64, oc, sc * P : (sc + 1) * P],
                            rhs=qT[base : base + 64, oc, :],
                            start=True,
                            stop=True,
                        )
                        nc.scalar.activation(
                            AT[:, sc, :, h],
                            ps[:],
                            mybir.ActivationFunctionType.Exp,
                            bias=bqks[:, sc, h : h + 1],
                        )

                # ---- phase 2b: out1T + Z ----
                for h in range(H):
                    pso = psSmall.tile([64, TB], F32, tag="out1T")
                    for sc in range(8):
                        nc.tensor.matmul(
                            pso[:],
                            lhsT=vS[:, sc, h * DH : (h + 1) * DH],
                            rhs=AT[:, sc, :, h],
                            start=(sc == 0),
                            stop=(sc == 7),
                        )
                    nc.vector.tensor_copy(out=oT[0:64, :, h], in_=pso[:])
                    psz = psSmall.tile([1, TB], F32, tag="zrow")
                    for sc in range(8):
                        nc.tensor.matmul(
                            psz[:],
                            lhsT=onescol[:, :],
                            rhs=AT[:, sc, :, h],
                            start=(sc == 0),
                            stop=(sc == 7),
                        )
                    nc.vector.reciprocal(out=recipZ[:, h, :], in_=psz[:])

            # acts (kT/vS/qT) freed here.
            with tc.tile_pool(name="late", bufs=1) as late:
                WoPs = late.tile([P, H, D], BF16)
                nc.sync.dma_start(WoPs[:], WoP.rearrange("h p o -> p h o"))
                Wsb = late.tile([P, 64, 64], BF16)  # [4t x 32-stride heads, grp, j]
                WT = late.tile([P, 32, P], BF16)
                oh2 = late.tile([P, 2, 8, TBATCH, NJ], BF16)
                nc.vector.memset(oh2[:, :, :, :, 41:42], 1.0)
                outsb = late.tile([P, 2, D], F32)

                # ---- phase 2c: W histogram matmuls ----
                n_batches = TB // TBATCH
                gl_per_batch = TBATCH // 4
                for tb8 in range(n_batches):
                    buf = tb8 % 2
                    for sc in range(8):
                        nc.vector.tensor_tensor(
                            out=oh2[:, buf, sc, :, 0:41],
                            in0=rTs[
                                :, sc, tb8 * TBATCH : (tb8 + 1) * TBATCH, None
                            ].to_broadcast([P, TBATCH, 41]),
                            in1=iotaF[:],
                            op=mybir.AluOpType.is_equal,
                        )
                    for gl in range(gl_per_batch):
                        grp = tb8 * gl_per_batch + gl
                        psw = psW.tile([P, NJ], F32, tag="wps")
                        for sc in range(8):
                            for c in range(4):
                                tl = gl * 4 + c
                                nc.tensor.matmul(
                                    psw[32 * c : 32 * c + 16, :],
                                    lhsT=AT[:, sc, tb8 * TBATCH + tl, :],
                                    rhs=oh2[:, buf, sc, tl, :],
                                    start=(sc == 0),
                                    stop=(sc == 7),
                                    tile_position=(0, 32 * c),
                                )
                        nc.vector.tensor_copy(out=Wsb[:, grp, 0:NJ], in_=psw[:])

                # ---- phase 2e: W transpose (DMA), two 64-wide groups per 128-col
                # XBAR transpose; WT rows 0-63 = even group's j, 64-127 = odd's ----
                for g2 in range(32):
                    nc.sync.dma_start_transpose(
                        WT[:, g2, :],
                        Wsb[:, 2 * g2 : 2 * g2 + 2, :].rearrange("p a b -> p (a b)"),
                    )

                # ---- phase 2f/2g: assemble + normalize oT ----
                # WT[64*par + j, g2, 32*c + h] -> oT[64 + j, h, g2*8 + par*4 + c]
                for par in range(2):
                    for c in range(4):
                        nc.sync.dma_start(
                            out=oT[64:105, :, :].rearrange(
                                "p (g2 par c) hh -> p g2 par c hh", par=2, c=4
                            )[:, :, par, c, :],
                            in_=WT[64 * par : 64 * par + 41, :, 32 * c : 32 * c + 16],
                        )
                for h in range(H):
                    psb = psSc.tile([P, TB], F32, tag="scoresT")
                    nc.tensor.matmul(
                        psb[:],
                        lhsT=onesrow[:, :],
                        rhs=recipZ[:, h, :],
                        start=True,
                        stop=True,
                    )
                    nc.vector.tensor_tensor(
                        out=oT[0:105, :, h],
                        in0=oT[0:105, :, h],
                        in1=psb[0:105, :],
                        op=mybir.AluOpType.mult,
                    )
                    nc.vector.tensor_tensor(
                        out=oT[0:64, :, h],
                        in0=oT[0:64, :, h],
                        in1=bvs[:, h : h + 1].to_broadcast([64, TB]),
                        op=mybir.AluOpType.add,
                    )

                # ---- phase 3: fused output projection ----
                for tc_i in range(2):
                    for ocj in range(2):
                        pso = psProj.tile([P, 512], F32, tag="proj")
                        for h in range(H):
                            nc.tensor.matmul(
                                pso[:],
                                lhsT=oT[:, tc_i * P : (tc_i + 1) * P, h],
                                rhs=WoPs[:, h, ocj * 512 : (ocj + 1) * 512],
                                start=(h == 0),
                                stop=(h == H - 1),
                            )
                        nc.vector.tensor_copy(
                            out=outsb[:, tc_i, ocj * 512 : (ocj + 1) * 512], in_=pso[:]
                        )
                nc.sync.dma_start(
                    out=out.rearrange("(tc p) o -> p tc o", p=P), in_=outsb[:]
                )


    nc.compile()
    return nc


_NC = None
_last_in_maps = None


def _get_nc():
    global _NC
    if _NC is None:
        _NC = build_bass()
    return _NC


def _prep_in_maps(hidden_states, relation_inputs, Wq, bq, Wk, bk, Wv, bv, Wo, bo, rel_emb):
    hidden_states = np.asarray(hidden_states, dtype=np.float32)
    relation_inputs = np.asarray(relation_inputs)
    scale = DH ** -0.5
    bf = ml_dtypes.bfloat16

    WqTs = (np.asarray(Wq, np.float32).T * scale).astype(bf)
    WkT = np.asarray(Wk, np.float32).T.astype(bf)
    WvT = np.asarray(Wv, np.float32).T.astype(bf)
    Wo = np.asarray(Wo, np.float32)
    E = np.asarray(rel_emb, np.float32)

    WoP = np.zeros((H, P, D), np.float32)
    for h in range(H):
        Wo_h = Wo[:, h * DH : (h + 1) * DH]  # [D, 64]
        WoP[h, 0:64, :] = Wo_h.T
        WoP[h, 64:105, :] = E @ Wo_h.T
    WoP[0, 105, :] = np.asarray(bo, np.float32)
    WoP = WoP.astype(bf)

    # bqk[s, h] = k_h[s] . (bq_h * scale) = (hs_b @ Wk_h.T @ bq_h*scale)[s]
    bqs = np.asarray(bq, np.float32) * scale
    wb = np.zeros((D, H), np.float32)
    for h in range(H):
        wb[:, h] = np.asarray(Wk, np.float32)[h * DH : (h + 1) * DH, :].T @ bqs[
            h * DH : (h + 1) * DH
        ]
    bvTa = np.asarray(bv, np.float32).reshape(H, DH).T.astype(bf)

    in_maps = []
    for core in range(N_CORES):
        b, tb = core // 4, core % 4
        hs_b = hidden_states[b]
        hsT_b = np.ascontiguousarray(hs_b.T).astype(bf)
        hsTq = np.ascontiguousarray(hs_b.T[:, tb * TB : (tb + 1) * TB]).astype(bf)
        rT_c = (
            np.ascontiguousarray(relation_inputs[b, tb * TB : (tb + 1) * TB, :].T)
            .astype(np.float32)
            .astype(bf)
        )
        bqk_c = (hs_b @ wb).astype(np.float32)
        in_maps.append(
            dict(
                hsT=hsT_b,
                hsTq=hsTq,
                rT=rT_c,
                WqT=WqTs,
                WkT=WkT,
                WvT=WvT,
                WoP=WoP,
                bqk=bqk_c,
                bvT=bvTa,
            )
        )
    return in_maps


def kernel(hidden_states, relation_inputs, Wq, bq, Wk, bk, Wv, bv, Wo, bo, rel_emb):
    global _last_in_maps
    in_maps = _prep_in_maps(
        hidden_states, relation_inputs, Wq, bq, Wk, bk, Wv, bv, Wo, bo, rel_emb
    )
    _last_in_maps = in_maps
    nc = _get_nc()
    res = bass_utils.run_bass_kernel_spmd(nc, in_maps, core_ids=list(range(N_CORES)))
    outs = [np.asarray(r["out"], np.float32) for r in res.results]
    full = np.empty((B, T, D), np.float32)
    for core in range(N_CORES):
        b, tb = core // 4, core % 4
        full[b, tb * TB : (tb + 1) * TB, :] = outs[core]
    return full



# revision 8
# speedup vs baseline: 2.0168x; 2.0168x over previous
"""BartCustomAttention Trainium2 kernel (v2).

Sharding: 8 cores = batch(2) x t-block(4, 256 query rows each). Each core
computes all 16 heads for its 256 query rows; k/v projections for its batch
element are computed redundantly on the 4 cores sharing it.

Relation-value term: out2[h,t,:] = sum_s attn[h,t,s] * E[r[t,s],:] = W @ E
with W the attn-weighted histogram of relation codes. Histogram matmuls use
the [42j, 16h] orientation (stationary = one-hot, moving = attn column), two
t's per PSUM tile via column tiling, so W lands j-on-partitions: no
transposes, and the ones row (j=41) yields Z = sum_s attn for free, which
becomes the constant-1 bias row after normalization.  W @ (E @ Wo_h.T) is
folded into the output projection via a host-packed weight.

One-hot matrices are built on DVE in the [P, 41j, t] orientation whose
operands are all packed 2-byte last-dim APs (broadcast only on the middle
dim), enabling the DVE 2x perf mode.  PSUM->SBUF evictions run on gpsimd
(Pool); exp and the W eviction on the scalar (Act) engine; scores / V proj /
histogram matmuls are software-pipelined on PE per s-chunk.

Pools are managed explicitly on the two allocator sides so the big
transients (weight windows, exp(scores), WoPs) can time-share SBUF with
non-nested lifetimes.  Softmax skips max-subtraction (scores are O(6)).
"""

import sys

if "/opt/trn_rl_repo" not in sys.path:
    sys.path.insert(0, "/opt/trn_rl_repo")

import numpy as np
import ml_dtypes

import concourse.bass as bass
from concourse import bacc
import concourse.mybir as mybir
import concourse.tile as tile
from concourse import bass_utils

B, T, D, H, DH = 2, 1024, 1024, 16, 64
NJ = 42  # 41 relation bins + ones row (Z)
TB = T // 4  # 256 query rows per core
P = 128
N_CORES = 8
OHC = 64  # t-columns per one-hot DVE op
OHT = 128  # t-columns per one-hot buffer tile (half of TB)

F32 = mybir.dt.float32
BF16 = mybir.dt.bfloat16
I32 = mybir.dt.int32


def build_bass(with_bias: bool):
    nc = bacc.Bacc(None, target_bir_lowering=False)

    hsT = nc.dram_tensor("hsT", [D, T], BF16, kind="ExternalInput")
    hsTq = nc.dram_tensor("hsTq", [D, TB], BF16, kind="ExternalInput")
    rT = nc.dram_tensor("rT", [T, TB], BF16, kind="ExternalInput")
    WqT = nc.dram_tensor("WqT", [D, D], BF16, kind="ExternalInput")
    WkT = nc.dram_tensor("WkT", [D, D], BF16, kind="ExternalInput")
    WvT = nc.dram_tensor("WvT", [D, D], BF16, kind="ExternalInput")
    WoP = nc.dram_tensor("WoP", [H, P, D], BF16, kind="ExternalInput")
    if with_bias:
        bqk = nc.dram_tensor("bqk", [T, H], F32, kind="ExternalInput")
        bvT = nc.dram_tensor("bvT", [DH, H], BF16, kind="ExternalInput")
    out = nc.dram_tensor("out", [TB, D], F32, kind="ExternalOutput")

    with tile.TileContext(nc) as tc:
        persist = tc.alloc_tile_pool(name="persist", bufs=1, side="left")
        rTs = persist.tile([P, 8, TB], BF16)
        nc.sync.dma_start(rTs[:], rT.rearrange("(sc p) t -> p sc t", p=P))
        iotaI = persist.tile([P, 41], I32)
        nc.gpsimd.iota(iotaI[:], pattern=[[1, 41]], base=0, channel_multiplier=0)
        iotaF = persist.tile([P, 41, OHC], BF16)
        nc.vector.tensor_copy(
            out=iotaF[:], in_=iotaI[:, :, None].to_broadcast([P, 41, OHC])
        )
        onesrow = persist.tile([1, P], BF16)
        nc.vector.memset(onesrow[:], 1.0)
        Wsb = persist.tile([P, TB // 2, H], BF16)
        if with_bias:
            bqks = persist.tile([P, 8, H], F32)
            nc.sync.dma_start(bqks[:], bqk.rearrange("(sc p) h -> p sc h", p=P))
            bvs = persist.tile([DH, H], BF16)
            nc.sync.dma_start(bvs[:], bvT[:, :])

        # ---- one-hot tiles (DVE queue; throttled by pool rotation) ----
        ohpool = tc.alloc_tile_pool(name="ohpool", bufs=4, side="left")
        oh = []
        for sc in range(8):
            for hf in range(2):
                t_ = ohpool.tile([P, NJ, OHT], BF16, name=f"oh{sc}_{hf}", tag="oh")
                oh.append(t_)
                nc.vector.memset(t_[:, 41:42, :], 1.0)
                for c in range(OHT // OHC):
                    t0 = hf * OHT + c * OHC
                    nc.vector.tensor_tensor(
                        out=t_[:, 0:41, c * OHC : (c + 1) * OHC],
                        in0=rTs[:, sc, None, t0 : t0 + OHC].to_broadcast(
                            [P, 41, OHC]
                        ),
                        in1=iotaF[:],
                        op=mybir.AluOpType.is_equal,
                    )

        vpool = tc.alloc_tile_pool(name="vpool", bufs=1, side="left")
        vS = vpool.tile([P, 8, D], BF16)

        psWp = tc.alloc_tile_pool(name="psW", bufs=1, side="right", space="PSUM")
        # W accumulator: pair pr=(t//2), parity on partition group 0/64
        psw = psWp.tile([P, TB // 2, H], F32)

        def emit_w(sc):
            for pr in range(TB // 2):
                for par in range(2):
                    t = 2 * pr + par
                    nc.tensor.matmul(
                        psw[64 * par : 64 * par + NJ, pr, :],
                        lhsT=oh[2 * sc + t // OHT][:, :, t % OHT],
                        rhs=AT[:, sc, :, t],
                        start=(sc == 0),
                        stop=(sc == 7),
                        tile_position=(0, 64 * par),
                    )

        psProjp = tc.alloc_tile_pool(name="psProj", bufs=2, side="left", space="PSUM")
        kqwinB = tc.alloc_tile_pool(name="kqwinB", bufs=1, side="left")
        kT = kqwinB.tile([P, 8, T], BF16)
        qT = kqwinB.tile([P, 8, TB], BF16)
        hsTs = kqwinB.tile([P, 8, T], BF16)
        Wv_s = kqwinB.tile([P, 8, D], BF16)
        kqwinA = tc.alloc_tile_pool(name="kqwinA", bufs=1, side="right")
        hsTqs = kqwinA.tile([P, 8, TB], BF16)
        Wk_s = kqwinA.tile([P, 8, D], BF16)
        Wq_s = kqwinA.tile([P, 8, D], BF16)
        psScp = tc.alloc_tile_pool(name="psSc", bufs=2, side="right", space="PSUM")

        hsT_r = hsT.rearrange("(ic p) s -> p ic s", p=P)
        WkT_r = WkT.rearrange("(ic p) o -> p ic o", p=P)
        for ic in range(8):
            nc.sync.dma_start(hsTs[:, ic, :], hsT_r[:, ic, :])
            nc.sync.dma_start(Wk_s[:, ic, :], WkT_r[:, ic, :])
        nc.sync.dma_start(hsTqs[:], hsTq.rearrange("(ic p) t -> p ic t", p=P))
        nc.sync.dma_start(Wq_s[:], WqT.rearrange("(ic p) o -> p ic o", p=P))
        nc.sync.dma_start(Wv_s[:], WvT.rearrange("(ic p) o -> p ic o", p=P))

        # ---- K projection ----
        for oc in range(8):
            for n in range(2):
                ps = psProjp.tile([P, 512], F32, tag="proj")
                for ic in range(8):
                    nc.tensor.matmul(
                        ps[:],
                        lhsT=Wk_s[:, ic, oc * P : (oc + 1) * P],
                        rhs=hsTs[:, ic, n * 512 : (n + 1) * 512],
                        start=(ic == 0),
                        stop=(ic == 7),
                    )
                nc.gpsimd.tensor_copy(out=kT[:, oc, n * 512 : (n + 1) * 512], in_=ps[:])
        # ---- Q projection ----
        for oc in range(8):
            ps = psProjp.tile([P, 512], F32, tag="proj")
            for ic in range(8):
                nc.tensor.matmul(
                    ps[:, :TB],
                    lhsT=Wq_s[:, ic, oc * P : (oc + 1) * P],
                    rhs=hsTqs[:, ic, :],
                    start=(ic == 0),
                    stop=(ic == 7),
                )
            nc.gpsimd.tensor_copy(out=qT[:, oc, :], in_=ps[:, :TB])

        kqwinA.release()
        ATpool = tc.alloc_tile_pool(name="ATpool", bufs=1, side="right")
        AT = ATpool.tile([P, 8, H, TB], BF16)

        # ---- interleaved scores / V projection / W histogram ----
        vq = [(sc2, n) for n in range(2) for sc2 in range(8)]
        for sc in range(8):
            for hp in range(8):
                ps = psScp.tile([P, 512], F32, tag="sc")
                for k2 in range(2):
                    h = 2 * hp + k2
                    base = 64 * (h % 2)
                    nc.tensor.matmul(
                        ps[:, k2 * TB : (k2 + 1) * TB],
                        lhsT=kT[base : base + 64, h // 2, sc * P : (sc + 1) * P],
                        rhs=qT[base : base + 64, h // 2, :],
                        start=True,
                        stop=True,
                    )
                if with_bias:
                    for k2 in range(2):
                        h = 2 * hp + k2
                        nc.scalar.activation(
                            AT[:, sc, h, :],
                            ps[:, k2 * TB : (k2 + 1) * TB],
                            mybir.ActivationFunctionType.Exp,
                            bias=bqks[:, sc, h : h + 1],
                        )
                else:
                    nc.scalar.activation(
                        AT[:, sc, 2 * hp : 2 * hp + 2, :].rearrange("p h t -> p (h t)"),
                        ps[:],
                        mybir.ActivationFunctionType.Exp,
                    )
            for sc2, n in vq[2 * sc : 2 * sc + 2]:
                ps = psProjp.tile([P, 512], F32, tag="proj")
                for ic in range(8):
                    nc.tensor.matmul(
                        ps[:],
                        lhsT=hsTs[:, ic, sc2 * P : (sc2 + 1) * P],
                        rhs=Wv_s[:, ic, n * 512 : (n + 1) * 512],
                        start=(ic == 0),
                        stop=(ic == 7),
                    )
                nc.gpsimd.tensor_copy(
                    out=vS[:, sc2, n * 512 : (n + 1) * 512], in_=ps[:]
                )
            if sc >= 1:
                emit_w(sc - 1)

        psScp.release()
        kqwinB.release()

        lateA = tc.alloc_tile_pool(name="lateA", bufs=1, side="left")
        oT = lateA.tile([P, TB, H], BF16)
        nc.vector.memset(oT[106:128, :, :], 0.0)
        recipZ = lateA.tile([1, TB, H], BF16)
        zeven = lateA.tile([1, TB // 2, H], BF16)
        zodd = lateA.tile([1, TB // 2, H], BF16)
        outsb = lateA.tile([P, 2, D], F32)

        psSmp = tc.alloc_tile_pool(name="psSm", bufs=2, side="left", space="PSUM")
        emit_w(7)
        # ---- out1 = attn @ v ----
        for h in range(H):
            pso = psSmp.tile([64, TB], F32, tag="out1")
            for sc in range(8):
                nc.tensor.matmul(
                    pso[:],
                    lhsT=vS[:, sc, h * DH : (h + 1) * DH],
                    rhs=AT[:, sc, h, :],
                    start=(sc == 0),
                    stop=(sc == 7),
                )
            nc.gpsimd.tensor_copy(out=oT[0:64, :, h], in_=pso[:])
            if with_bias:
                nc.vector.tensor_tensor(
                    out=oT[0:64, :, h],
                    in0=oT[0:64, :, h],
                    in1=bvs[:, h : h + 1].to_broadcast([64, TB]),
                    op=mybir.AluOpType.add,
                )

        ATpool.release()

        # ---- evict W + Z ----
        nc.scalar.activation(
            Wsb[0:NJ, :, :], psw[0:NJ, :, :], mybir.ActivationFunctionType.Copy
        )
        nc.scalar.activation(
            Wsb[64 : 64 + NJ, :, :],
            psw[64 : 64 + NJ, :, :],
            mybir.ActivationFunctionType.Copy,
        )
        nc.sync.dma_start(out=oT[64:106, 0:TB:2, :], in_=Wsb[0:NJ, :, :])
        nc.sync.dma_start(out=oT[64:106, 1:TB:2, :], in_=Wsb[64 : 64 + NJ, :, :])
        nc.sync.dma_start(out=zeven[:], in_=Wsb[41:42, :, :])
        nc.sync.dma_start(out=zodd[:], in_=Wsb[105:106, :, :])
        with nc.allow_low_precision(reason="1/Z in bf16; rel tolerance is 2e-2"):
            nc.vector.reciprocal(out=recipZ[:, 0:TB:2, :], in_=zeven[:])
            nc.vector.reciprocal(out=recipZ[:, 1:TB:2, :], in_=zodd[:])

        psSmp.release()
        psProjp.release()
        psWp.release()

        WoPpool = tc.alloc_tile_pool(name="WoPpool", bufs=1, side="right")
        WoPs = WoPpool.tile([P, H, D], BF16)
        WoP_r = WoP.rearrange("h p o -> p h o")
        for hq in range(4):
            nc.sync.dma_start(
                WoPs[:, 4 * hq : 4 * hq + 4, :], WoP_r[:, 4 * hq : 4 * hq + 4, :]
            )

        psBp = tc.alloc_tile_pool(name="psB", bufs=2, side="left", space="PSUM")
        psOutp = tc.alloc_tile_pool(name="psOut", bufs=1, side="left", space="PSUM")
        pouts = [
            psOutp.tile([P, 512], F32, name=f"po{i}", tag=f"po{i}") for i in range(4)
        ]
        # ---- normalize + output projection, pipelined over h ----
        for h in range(H):
            psb = psBp.tile([P, TB], F32, tag="bcast")
            nc.tensor.matmul(
                psb[:], lhsT=onesrow[:], rhs=recipZ[:, :, h], start=True, stop=True
            )
            nc.vector.tensor_tensor(
                out=oT[0:106, :, h],
                in0=oT[0:106, :, h],
                in1=psb[0:106, :],
                op=mybir.AluOpType.mult,
            )
            for tc_i in range(2):
                for ocj in range(2):
                    nc.tensor.matmul(
                        pouts[2 * tc_i + ocj][:],
                        lhsT=oT[:, tc_i * P : (tc_i + 1) * P, h],
                        rhs=WoPs[:, h, ocj * 512 : (ocj + 1) * 512],
                        start=(h == 0),
                        stop=(h == H - 1),
                    )
        for tc_i in range(2):
            for ocj in range(2):
                nc.scalar.activation(
                    outsb[:, tc_i, ocj * 512 : (ocj + 1) * 512],
                    pouts[2 * tc_i + ocj][:],
                    mybir.ActivationFunctionType.Copy,
                )
        nc.sync.dma_start(
            out=out.rearrange("(tc p) o -> p tc o", p=P), in_=outsb[:]
        )

        psOutp.release()
        psBp.release()
        WoPpool.release()
        lateA.release()
        vpool.release()
        ohpool.release()
        persist.release()

    nc.compile()
    return nc


_NC = {}
_last_in_maps = None


def _get_nc(with_bias: bool = False):
    if with_bias not in _NC:
        _NC[with_bias] = build_bass(with_bias)
    return _NC[with_bias]


def _prep_in_maps(hidden_states, relation_inputs, Wq, bq, Wk, bk, Wv, bv, Wo, bo,
                  rel_emb, with_bias):
    hidden_states = np.asarray(hidden_states, dtype=np.float32)
    relation_inputs = np.asarray(relation_inputs)
    scale = DH ** -0.5
    bf = ml_dtypes.bfloat16

    WqTs = (np.asarray(Wq, np.float32).T * scale).astype(bf)
    WkT = np.asarray(Wk, np.float32).T.astype(bf)
    WvT = np.asarray(Wv, np.float32).T.astype(bf)
    Wo = np.asarray(Wo, np.float32)
    E = np.asarray(rel_emb, np.float32)

    WoP = np.zeros((H, P, D), np.float32)
    for h in range(H):
        Wo_h = Wo[:, h * DH : (h + 1) * DH]  # [D, 64]
        WoP[h, 0:64, :] = Wo_h.T
        WoP[h, 64:105, :] = E @ Wo_h.T
    WoP[0, 105, :] = np.asarray(bo, np.float32)
    WoP = WoP.astype(bf)

    if with_bias:
        # bqk[s, h] = k_h[s] . (bq_h * scale)
        bqs = np.asarray(bq, np.float32) * scale
        wb = np.zeros((D, H), np.float32)
        for h in range(H):
            wb[:, h] = np.asarray(Wk, np.float32)[h * DH : (h + 1) * DH, :].T @ bqs[
                h * DH : (h + 1) * DH
            ]
        bvTa = np.asarray(bv, np.float32).reshape(H, DH).T.astype(bf)

    in_maps = []
    for core in range(N_CORES):
        b, tb = core // 4, core % 4
        hs_b = hidden_states[b]
        hsT_b = np.ascontiguousarray(hs_b.T).astype(bf)
        hsTq_c = np.ascontiguousarray(hs_b.T[:, tb * TB : (tb + 1) * TB]).astype(bf)
        rT_c = (
            np.ascontiguousarray(relation_inputs[b, tb * TB : (tb + 1) * TB, :].T)
            .astype(np.float32)
            .astype(bf)
        )
        m = dict(
            hsT=hsT_b, hsTq=hsTq_c, rT=rT_c,
            WqT=WqTs, WkT=WkT, WvT=WvT, WoP=WoP,
        )
        if with_bias:
            m["bqk"] = (hs_b @ wb).astype(np.float32)
            m["bvT"] = bvTa
        in_maps.append(m)
    return in_maps


def kernel(hidden_states, relation_inputs, Wq, bq, Wk, bk, Wv, bv, Wo, bo, rel_emb):
    global _last_in_maps
    with_bias = not (
        np.all(np.asarray(bq) == 0) and np.all(np.asarray(bv) == 0)
    )
    in_maps = _prep_in_maps(
        hidden_states, relation_inputs, Wq, bq, Wk, bk, Wv, bv, Wo, bo, rel_emb,
        with_bias,
    )
    _last_in_maps = in_maps
    nc = _get_nc(with_bias)
    res = bass_utils.run_bass_kernel_spmd(nc, in_maps, core_ids=list(range(N_CORES)))
    outs = [np.asarray(r["out"], np.float32) for r in res.results]
    full = np.empty((B, T, D), np.float32)
    for core in range(N_CORES):
        b, tb = core // 4, core % 4
        full[b, tb * TB : (tb + 1) * TB, :] = outs[core]
    return full
